# revision 20
# baseline (speedup 1.0000x reference)
"""Correlation cost-volume kernel (max_displacement=4) for 8 Trainium2 cores.

Problem: in1, in2: [B=8, C=256, H=128, W=128] f32.
out[b, dy*9+dx, h, w] = sum_c in1[b,c,h,w] * pad(in2)[b, c, h+dy, w+dx]
(pad = 4 zeros on each spatial side), output [8, 81, 128, 128] f32.

Strategy (data-parallel, one batch sample per core):
  2D-tiled gram with tall tiles and partition-split output windows.  Each
  matmul tile covers 16h x 8w in1 pixels (M = 128 PSUM partitions, pixel
  (mh,mw) on partition m = 8*mh+mw) against its 24 x 16 halo of padded in2
  (N = 384 gram columns n = rh*16+rw, C contracted as two K=128 tiles).
  The 81-entry band for pixel (mh,mw) lives at n = (mh+dy)*16 + (mw+dx),
  i.e. inside [16*mh, 16*mh+144) -- so the LOWER 64 partitions (mh<8) only
  ever need columns [0,256) and the UPPER 64 (mh>=8) only [128,384).  Two
  partition-range PSUM->SBUF copies per tile store exactly those 256-column
  halves at identical byte offsets, giving a partition-uniform [128, 256]
  tile that ONE full-width (all 16 SDMA ports) DMA per 16-row stripe ships
  to HBM: 8.4 MB total vs 12.6 MB for the full gram.  The host gathers the
  per-pixel 81-band from the 256-column windows with numpy for free.

  DMA plumbing (the real bottlenecks found by tracing): HWDGE descriptor
  generation caps a stream of 1 KB-descriptor loads at ~180 GB/s, so in2
  ships in 16-row chunks (4.3 KB descriptors) and in1 in half-stripe chunks
  (4 KB descriptors).  Reads are split across BOTH HWDGE rings (in2 on
  sync, in1 on scalar) and each stripe's window write is issued as soon as
  its copies land, alternating rings, so the ~2us HBM-write receipts and
  the write data ride inside the read phase instead of draining after it.
"""

import ml_dtypes
import numpy as np

import concourse.bass as bass
import concourse.bacc as bacc
import concourse.mybir as mybir
from concourse.bass_utils import run_bass_kernel_spmd
from concourse.tile import TileContext

B, C, H, W = 8, 256, 128, 128
D = 4
ND = 2 * D + 1  # 9 displacements per axis
HP = H + 2 * D  # 136 padded rows
WP = W + 2 * D  # 136 padded cols
KT = C // 128  # 2 contraction tiles
TH, TW = 16, 8  # matmul tile = 16h x 8w pixels (128 = PSUM partition dim)
NHT, NWT = H // TH, W // TW  # 8 row-stripes x 16 tiles each
RH, RW = TH + 2 * D, TW + 2 * D  # 24 x 16 halo region
NR = RH * RW  # 384 gram columns per tile
NT = NHT * NWT  # 128 tiles
WC = 256  # stored window columns per pixel (half-split)
NAC = 2 * NHT  # 16 half-stripe in1 chunks

OUT_DT = mybir.dt.bfloat16
_OUT_NP = ml_dtypes.bfloat16

_CACHED_NC = None


def _build_nc():
    bf16 = mybir.dt.bfloat16

    nc = bacc.Bacc()
    # in1 as [c][chunk=2*ht+half][kt][w(8)][m=mh*8+mw]; in2 zero-padded in w
    # only, as [c][kt][h(128)][wp] -- the 8 pad rows are memset on-chip
    in1_t = nc.declare_dram_parameter("in1_t", [128, NAC, KT, 8, 128], bf16, isOutput=False)
    # kt folded inside rows: one 8.7KB descriptor per partition per chunk
    in2_p = nc.declare_dram_parameter("in2_p", [128, H, KT, WP], bf16, isOutput=False)
    # per-stripe tile-grams: [ht][m][wt][384]
    out_c = nc.declare_dram_parameter(
        "out_c", [NHT, 128, NWT, NR], OUT_DT, isOutput=True
    )

    with TileContext(nc) as tc:
        with (
            tc.tile_pool(name="bpool", bufs=1) as bpool,
            tc.tile_pool(name="apool", bufs=6) as apool,
            tc.tile_pool(name="spool", bufs=1) as spool,
            tc.tile_pool(name="psum", bufs=8, space="PSUM") as ppool,
        ):
            # whole padded in2 sample resident in SBUF (72.25KB/partition)
            b_s = bpool.tile([128, HP, KT, WP], bf16)
            # zero the 4 pad rows top+bottom (pad cols ship from DRAM)
            nc.gpsimd.memset(b_s[:, 0:D], 0.0)
            nc.gpsimd.memset(b_s[:, D + H :], 0.0)
            # full tile-grams: [m][tile][384] (96KB/partition)
            st = spool.tile([128, NT, NR], OUT_DT)

            def load_b(r0, nr):  # rows [r0, r0+nr) of the interior
                return nc.sync.dma_start(
                    out=b_s[:, D + r0 : D + r0 + nr],
                    in_=in2_p[:, r0 : r0 + nr],
                )

            a_tiles = {}

            def load_a(c, eng):  # half-stripe chunk -> 4KB descriptors
                a_t = apool.tile([128, KT, 8, 128], bf16, tag="a")
                a_tiles[c] = a_t
                return eng.dma_start(out=a_t, in_=in1_t[:, c])

            # reads split across both HWDGE rings.  The first matmul gates
            # on a0 + in2 rows [0,28), so a0 leads the sync ring followed by
            # small 8-row in2 chunks; everything else streams behind.
            load_a(0, nc.sync)
            for r0 in (0, 8, 16, 24):
                load_b(r0, 8)
            load_a(1, nc.scalar)
            for c in range(2, NAC):
                load_a(c, nc.scalar)
            for r0 in range(32, H, 16):
                load_b(r0, 16)

            for ht in range(NHT):
                r0 = TH * ht
                for wt in range(NWT):
                    w0 = TW * wt
                    t = ht * NWT + wt
                    ps = ppool.tile([128, NR], mybir.dt.float32, name="ps", tag="ps")
                    ac = 2 * ht + wt // 8
                    for kt in range(KT):
                        nc.tensor.matmul(
                            ps,
                            a_tiles[ac][:, kt, wt % 8, :],
                            b_s[:, r0 : r0 + RH, kt, w0 : w0 + RW],
                            start=(kt == 0),
                            stop=(kt == KT - 1),
                        )
                    # one cheap full-gram copy per tile (PSUM drain on the
                    # two PSUM-capable engines is the pipeline's scarce
                    # resource: 384 elems fits the stripe cadence, 512 not)
                    if t % 2 == 0:
                        nc.vector.tensor_copy(st[:, t, :], ps)
                    else:
                        nc.scalar.copy(st[:, t, :], ps)
                # ship the whole stripe-gram as one full-width DMA (128
                # descriptors of 12KB -- descgen stays off the copy engines'
                # critical path) on the otherwise-idle sync ring; each write
                # fires as soon as its stripe's copies land, so the 12.6MB
                # of writes ride the read/compute phase.  The final stripe
                # splits across both rings to halve the end-of-kernel drain.
                if ht < NHT - 1:
                    nc.sync.dma_start(
                        out=out_c[ht], in_=st[:, 16 * ht : 16 * ht + 16, :]
                    )
                else:
                    nc.sync.dma_start(
                        out=out_c[ht, :, 0:8], in_=st[:, 16 * ht : 16 * ht + 8, :]
                    )
                    nc.scalar.dma_start(
                        out=out_c[ht, :, 8:16], in_=st[:, 16 * ht + 8 : 16 * ht + 16, :]
                    )

    nc.compile()
    return nc


def _get_nc():
    global _CACHED_NC
    if _CACHED_NC is None:
        _CACHED_NC = _build_nc()
    return _CACHED_NC


def _make_in_maps(in1: np.ndarray, in2: np.ndarray):
    in_maps = []
    for b in range(B):
        # [C,H,W] -> [c(128), chunk=2*ht+half, kt, w(8), m=mh*8+mw]
        a = (
            in1[b]
            .astype(ml_dtypes.bfloat16)
            .reshape(KT, 128, NHT, TH, 2, 8, TW)
            .transpose(1, 2, 4, 0, 5, 3, 6)
            .reshape(128, NAC, KT, 8, 128)
        )
        p = np.zeros((KT, 128, H, WP), ml_dtypes.bfloat16)
        p[:, :, :, D : D + W] = in2[b].astype(ml_dtypes.bfloat16).reshape(
            KT, 128, H, W
        )
        in_maps.append(
            {
                "in1_t": np.ascontiguousarray(a),
                # [c, h, kt, wp]
                "in2_p": np.ascontiguousarray(p.transpose(1, 2, 0, 3)),
            }
        )
    return in_maps


_M = np.arange(128)
_MH, _MW = _M >> 3, _M & 7
# gram column of band entry (dy,dx) for partition m: n = (mh+dy)*16 + (mw+dx)
_QIDX = (
    16 * _MH[:, None, None]
    + _MW[:, None, None]
    + 16 * np.arange(ND)[None, :, None]
    + np.arange(ND)[None, None, :]
).reshape(128, 1, ND * ND)


def _extract_band(g: np.ndarray) -> np.ndarray:
    """[NHT, 128, NWT, NR] stripe tile-grams -> [81, H, W] cost volume."""
    win = (
        g.reshape(NHT, 128, NWT, NR)
        .transpose(1, 0, 2, 3)
        .reshape(128, NT, NR)
        .astype(np.float32)
    )
    band = np.take_along_axis(win, _QIDX, axis=2)  # [m, t, 81]
    # [m=(mh,mw), t=(ht,wt), d] -> [d, ht, mh, wt, mw] -> [81, H, W]
    band = band.reshape(TH, TW, NHT, NWT, ND * ND).transpose(4, 2, 0, 3, 1)
    return np.ascontiguousarray(band).reshape(ND * ND, H, W)


def kernel(**inputs) -> np.ndarray:
    in1 = np.ascontiguousarray(np.asarray(inputs["in1"], dtype=np.float32))
    in2 = np.ascontiguousarray(np.asarray(inputs["in2"], dtype=np.float32))
    assert in1.shape == (B, C, H, W) and in2.shape == (B, C, H, W)

    nc = _get_nc()
    in_maps = _make_in_maps(in1, in2)
    res = run_bass_kernel_spmd(nc, in_maps, list(range(B)))

    outs = [_extract_band(np.asarray(res.results[b]["out_c"])) for b in range(B)]
    return np.stack(outs).astype(np.float32)


# revision 21
# speedup vs baseline: 1.0792x; 1.0792x over previous
"""Correlation cost-volume kernel (max_displacement=4) for 8 Trainium2 cores.

Problem: in1, in2: [B=8, C=256, H=128, W=128] f32.
out[b, dy*9+dx, h, w] = sum_c in1[b,c,h,w] * pad(in2)[b, c, h+dy, w+dx]
(pad = 4 zeros on each spatial side), output [8, 81, 128, 128] f32.

Strategy (data-parallel, one batch sample per core):
  2D-tiled gram.  Each matmul tile covers an 8h x 16w block of in1 pixels
  (M = 128 PSUM partitions, pixel (mh,mw) on partition m = 16*mh+mw)
  against its 16 x 24 halo of padded in2 (N = 384 gram columns
  n = rh*24 + rw, C contracted as two K=128 tiles accumulated in PSUM).
  The 8x16 tile shape keeps the moving-operand AP at 48B inner runs, which
  measures ~20% faster on TensorE than the transposed 16x8 tiling.  The
  band entry for pixel (mh,mw) sits at gram column (mh+dy)*24 + (mw+dx),
  a per-partition sheared offset no DMA access pattern can express, so the
  device ships full tile-grams (bf16) and the host slices the 81-entry
  band per pixel with a numpy gather for free.

  DMA plumbing (the bottlenecks found by tracing): HWDGE descriptor
  generation caps a stream of ~1KB-descriptor loads at ~180 GB/s, so in2
  ships kt-folded in 16-row chunks (one 8.7KB descriptor per partition)
  and in1 in per-stripe chunks (4KB descriptors); reads split across BOTH
  HWDGE rings (in2 on sync, in1 on scalar) and sustain ~420 GB/s.  Each
  stripe-pair's gram ships as one full-width 1.57MB DMA (128 descriptors
  of 12KB) on the sync ring as soon as its copies land, so the 12.6MB of
  writes and their ~2us HBM receipts ride the read/compute phase instead
  of draining after it; the final two stripes split across both rings to
  halve the end-of-kernel drain.
"""

import ml_dtypes
import numpy as np

import concourse.bass as bass
import concourse.bacc as bacc
import concourse.mybir as mybir
from concourse.bass_utils import run_bass_kernel_spmd
from concourse.tile import TileContext

B, C, H, W = 8, 256, 128, 128
D = 4
ND = 2 * D + 1  # 9 displacements per axis
HP = H + 2 * D  # 136 padded rows
WP = W + 2 * D  # 136 padded cols
KT = C // 128  # 2 contraction tiles
TH, TW = 8, 16  # matmul tile = 8h x 16w pixels (128 = PSUM partition dim)
NHT, NWT = H // TH, W // TW  # 16 row-stripes x 8 tiles each
RH, RW = TH + 2 * D, TW + 2 * D  # 16 x 24 halo region
NR = RH * RW  # 384 gram columns per tile
NT = NHT * NWT  # 128 tiles

OUT_DT = mybir.dt.bfloat16
_OUT_NP = ml_dtypes.bfloat16

_CACHED_NC = None


def _build_nc():
    bf16 = mybir.dt.bfloat16

    nc = bacc.Bacc()
    # in1 as [c][ht][kt][wt][m=mh*16+mw]; in2 zero-padded in w only and
    # kt-folded inside rows, as [c][h(128)][kt][wp] -- the 8 pad rows are
    # memset on-chip
    in1_t = nc.declare_dram_parameter("in1_t", [128, NHT, KT, NWT, 128], bf16, isOutput=False)
    in2_p = nc.declare_dram_parameter("in2_p", [128, H, KT, WP], bf16, isOutput=False)
    # tile-grams, partition-major: [m][ht][wt][384]
    out_c = nc.declare_dram_parameter(
        "out_c", [128, NHT, NWT, NR], OUT_DT, isOutput=True
    )

    with TileContext(nc) as tc:
        with (
            tc.tile_pool(name="bpool", bufs=1) as bpool,
            tc.tile_pool(name="apool", bufs=6) as apool,
            tc.tile_pool(name="spool", bufs=1) as spool,
            tc.tile_pool(name="psum", bufs=8, space="PSUM") as ppool,
        ):
            # whole padded in2 sample resident in SBUF (72.25KB/partition)
            b_s = bpool.tile([128, HP, KT, WP], bf16)
            # zero the 4 pad rows top+bottom (pad cols ship from DRAM)
            nc.gpsimd.memset(b_s[:, 0:D], 0.0)
            nc.gpsimd.memset(b_s[:, D + H :], 0.0)
            # all 128 tile-grams: [m][tile][384] (96KB/partition)
            st = spool.tile([128, NT, NR], OUT_DT)

            def load_b(k):  # 16-row chunk k -> one 8.7KB desc per partition
                return nc.sync.dma_start(
                    out=b_s[:, D + 16 * k : D + 16 * k + 16],
                    in_=in2_p[:, 16 * k : 16 * k + 16],
                )

            a_tiles = {}

            def load_a(c):  # stripe chunk -> 4KB descriptors
                a_t = apool.tile([128, KT, NWT, 128], bf16, tag="a")
                a_tiles[c] = a_t
                return nc.scalar.dma_start(out=a_t, in_=in1_t[:, c])

            # reads split across both HWDGE rings: in2 on sync, in1 on
            # scalar, interleaved so stripe ht's inputs land proportionally
            ib = 0
            for ht in range(NHT):
                while ib < min(H // 16, (TH * ht + 11) // 16 + 1):
                    load_b(ib)
                    ib += 1
                load_a(ht)

            for ht in range(NHT):
                r0 = TH * ht
                for wt in range(NWT):
                    w0 = TW * wt
                    t = ht * NWT + wt
                    ps = ppool.tile([128, NR], mybir.dt.float32, name="ps", tag="ps")
                    for kt in range(KT):
                        nc.tensor.matmul(
                            ps,
                            a_tiles[ht][:, kt, wt, :],
                            b_s[:, r0 : r0 + RH, kt, w0 : w0 + RW],
                            start=(kt == 0),
                            stop=(kt == KT - 1),
                        )
                    # one cheap full-gram copy per tile; PSUM drain on the
                    # two PSUM-capable engines is the pipeline's scarce
                    # resource, so keep it at 384 contiguous elems/tile
                    if t % 2 == 0:
                        nc.vector.tensor_copy(st[:, t, :], ps)
                    else:
                        nc.scalar.copy(st[:, t, :], ps)
                # ship each stripe-PAIR as one full-width 1.57MB DMA (128
                # descriptors of 12KB -- descgen stays off the copy engines)
                # on the otherwise-idle sync ring, as soon as the copies
                # land; the last two stripes go as singles on both rings to
                # halve the end-of-kernel drain
                if ht % 2 == 1 and ht < NHT - 2:
                    k = ht // 2
                    nc.sync.dma_start(
                        out=out_c[:, 2 * k : 2 * k + 2],
                        in_=st[:, 16 * k : 16 * k + 16, :],
                    )
                elif ht == NHT - 2:
                    nc.sync.dma_start(
                        out=out_c[:, ht : ht + 1],
                        in_=st[:, NWT * ht : NWT * ht + NWT, :],
                    )
                elif ht == NHT - 1:
                    nc.scalar.dma_start(
                        out=out_c[:, ht : ht + 1],
                        in_=st[:, NWT * ht : NWT * ht + NWT, :],
                    )

    nc.compile()
    return nc


def _get_nc():
    global _CACHED_NC
    if _CACHED_NC is None:
        _CACHED_NC = _build_nc()
    return _CACHED_NC


def _make_in_maps(in1: np.ndarray, in2: np.ndarray):
    in_maps = []
    for b in range(B):
        # [C,H,W] -> [c(128), ht, kt, wt, m=mh*16+mw]
        a = (
            in1[b]
            .astype(ml_dtypes.bfloat16)
            .reshape(KT, 128, NHT, TH, NWT, TW)
            .transpose(1, 2, 0, 4, 3, 5)
            .reshape(128, NHT, KT, NWT, 128)
        )
        p = np.zeros((KT, 128, H, WP), ml_dtypes.bfloat16)
        p[:, :, :, D : D + W] = in2[b].astype(ml_dtypes.bfloat16).reshape(
            KT, 128, H, W
        )
        in_maps.append(
            {
                "in1_t": np.ascontiguousarray(a),
                # [c, h, kt, wp]
                "in2_p": np.ascontiguousarray(p.transpose(1, 2, 0, 3)),
            }
        )
    return in_maps


_M = np.arange(128)
_MH, _MW = _M >> 4, _M & 15
# gram column of band entry (dy,dx) for partition m: n = (mh+dy)*24 + (mw+dx)
_QIDX = (
    24 * _MH[:, None, None]
    + _MW[:, None, None]
    + 24 * np.arange(ND)[None, :, None]
    + np.arange(ND)[None, None, :]
).reshape(128, 1, ND * ND)


def _extract_band(g: np.ndarray) -> np.ndarray:
    """[128, NHT, NWT, NR] tile-grams -> [81, H, W] cost volume."""
    win = g.reshape(128, NT, NR).astype(np.float32)
    band = np.take_along_axis(win, _QIDX, axis=2)  # [m, t, 81]
    # [m=(mh,mw), t=(ht,wt), d] -> [d, ht, mh, wt, mw] -> [81, H, W]
    band = band.reshape(TH, TW, NHT, NWT, ND * ND).transpose(4, 2, 0, 3, 1)
    return np.ascontiguousarray(band).reshape(ND * ND, H, W)


def kernel(**inputs) -> np.ndarray:
    in1 = np.ascontiguousarray(np.asarray(inputs["in1"], dtype=np.float32))
    in2 = np.ascontiguousarray(np.asarray(inputs["in2"], dtype=np.float32))
    assert in1.shape == (B, C, H, W) and in2.shape == (B, C, H, W)

    nc = _get_nc()
    in_maps = _make_in_maps(in1, in2)
    res = run_bass_kernel_spmd(nc, in_maps, list(range(B)))

    outs = [_extract_band(np.asarray(res.results[b]["out_c"])) for b in range(B)]
    return np.stack(outs).astype(np.float32)
